# revision 4
# baseline (speedup 1.0000x reference)
"""VQ codebook nearest-neighbor kernel for Trainium2 (8 NeuronCores).

Problem: z_e (32,1024,64) f32, emb_table (8192,64) f32.
  dist2[t,v] = ||z_t||^2 - 2 z_t.e_v + ||e_v||^2
  w[t] = argmin_v dist2          (first index on ties)
  w_emb[t] = emb_table[w[t]]
Returns (w (32,1024) int32, w_emb (32,1024,64) f32).

Sharding: data-parallel over batch; core k handles batches [4k, 4k+4).
Per core: score[t,v] = 2 z.e_v - ||e_v||^2 (argmax == argmin dist) via
fp32 matmuls (K=65 incl. the -||e||^2 row) into PSUM, reduce-max on DVE,
ACT drains PSUM->SBUF, max_index recovers the argmax index exactly
(first-occurrence ties like jnp.argmin), then indirect DMA gathers
emb_table rows for w_emb.
"""
import sys

sys.path.insert(0, "/opt/trn_rl_repo")

import numpy as np

import concourse.bass as bass
import concourse.mybir as mybir
from concourse import bacc
from concourse.bass_utils import run_bass_kernel_spmd
from concourse.tile import TileContext

N_CORES = 8
BS, NT, D, V = 32, 1024, 64, 8192
TOK_PER_CORE = BS * NT // N_CORES          # 4096
P = 128                                    # tokens per tile (partitions)
N_TILES = TOK_PER_CORE // P                # 32
KAUG = D + 1                               # 65: z dims + ones row
QUARTER = 2048                             # psum quarter (4 banks)
N_Q = V // QUARTER                         # 4 quarters
CHUNK = 512                                # matmul free dim / psum bank
CPQ = QUARTER // CHUNK                     # chunks per quarter


def build_program(n_tiles=N_TILES, repeats=1):
    nc = bacc.Bacc("TRN2", target_bir_lowering=False)
    zt_d = nc.dram_tensor("zt", [KAUG, n_tiles * P], mybir.dt.float32,
                          kind="ExternalInput")
    b_d = nc.dram_tensor("baug", [KAUG, V], mybir.dt.float32,
                         kind="ExternalInput")
    emb_d = nc.dram_tensor("emb", [V, D], mybir.dt.float32,
                           kind="ExternalInput")
    w_d = nc.dram_tensor("w", [n_tiles * P, 1], mybir.dt.int32,
                         kind="ExternalOutput")
    wemb_d = nc.dram_tensor("wemb", [n_tiles * P, D], mybir.dt.float32,
                            kind="ExternalOutput")

    with TileContext(nc) as tc:
        with tc.tile_pool(name="bsb", bufs=1) as bsb, \
             tc.tile_pool(name="zsb", bufs=2) as zsb, \
             tc.tile_pool(name="ssb", bufs=2) as ssb, \
             tc.tile_pool(name="small", bufs=3) as small, \
             tc.tile_pool(name="wsb", bufs=3) as wsb, \
             tc.tile_pool(name="ps", bufs=2, space="PSUM") as ps:
            baug = bsb.tile([KAUG, V], mybir.dt.float32)
            nc.sync.dma_start(out=baug[:], in_=b_d[:])

            for t in [t for _ in range(repeats) for t in range(n_tiles)]:
                zt = zsb.tile([KAUG, P], mybir.dt.float32)
                nc.sync.dma_start(out=zt[:], in_=zt_d[:, t * P:(t + 1) * P])

                scores = ssb.tile([P, V], mybir.dt.float32)
                qmaxs = small.tile([P, N_Q], mybir.dt.float32)
                for q in range(N_Q):
                    pq = ps.tile([P, QUARTER], mybir.dt.float32)
                    for cc in range(CPQ):
                        c = q * CPQ + cc
                        nc.tensor.matmul(
                            out=pq[:, cc * CHUNK:(cc + 1) * CHUNK],
                            lhsT=zt[:],
                            rhs=baug[:, c * CHUNK:(c + 1) * CHUNK],
                            start=True, stop=True)
                    # chunk-max on DVE straight from PSUM
                    nc.vector.tensor_reduce(
                        out=qmaxs[:, q:q + 1], in_=pq[:],
                        axis=mybir.AxisListType.X, op=mybir.AluOpType.max)
                    # drain PSUM quarter to SBUF on ACT
                    nc.scalar.copy(out=scores[:, q * QUARTER:(q + 1) * QUARTER],
                                   in_=pq[:])

                gm8 = small.tile([P, 8], mybir.dt.float32)
                nc.vector.tensor_reduce(
                    out=gm8[:, 0:1], in_=qmaxs[:],
                    axis=mybir.AxisListType.X, op=mybir.AluOpType.max)
                nc.vector.tensor_copy(
                    out=gm8[:, 1:8],
                    in_=gm8[:, 0:1].to_broadcast([P, 7]))
                i8 = small.tile([P, 8], mybir.dt.uint32)
                nc.vector.max_index(out=i8[:], in_max=gm8[:], in_values=scores[:])

                w_i32 = wsb.tile([P, 1], mybir.dt.int32)
                nc.vector.tensor_copy(out=w_i32[:], in_=i8[:, 0:1])
                nc.sync.dma_start(out=w_d[t * P:(t + 1) * P, :], in_=w_i32[:])

                wemb = wsb.tile([P, D], mybir.dt.float32)
                nc.gpsimd.indirect_dma_start(
                    out=wemb[:], out_offset=None, in_=emb_d[:],
                    in_offset=bass.IndirectOffsetOnAxis(ap=w_i32[:, :1], axis=0))
                nc.sync.dma_start(out=wemb_d[t * P:(t + 1) * P, :], in_=wemb[:])

    nc.compile()
    return nc


def _prep_inputs(z_e, emb_table):
    """Host-side packing: per-core zT_aug shards + shared Baug."""
    z = np.ascontiguousarray(z_e, dtype=np.float32)
    E = np.ascontiguousarray(emb_table, dtype=np.float32)
    e_sq = (E * E).sum(axis=1, dtype=np.float32)
    baug = np.empty((KAUG, V), dtype=np.float32)
    baug[:D] = (2.0 * E).T
    baug[D] = -e_sq
    in_maps = []
    for k in range(N_CORES):
        zk = z.reshape(N_CORES, TOK_PER_CORE, D)[k]
        zt = np.empty((KAUG, TOK_PER_CORE), dtype=np.float32)
        zt[:D] = zk.T
        zt[D] = 1.0
        in_maps.append({"zt": zt, "baug": baug, "emb": E})
    return in_maps


_NC_CACHE = {}


def kernel(z_e, emb_table):
    if "nc" not in _NC_CACHE:
        _NC_CACHE["nc"] = build_program()
    nc = _NC_CACHE["nc"]
    in_maps = _prep_inputs(z_e, emb_table)
    res = run_bass_kernel_spmd(nc, in_maps, list(range(N_CORES)))
    w = np.concatenate([r["w"][:, 0] for r in res.results]).reshape(BS, NT)
    w_emb = np.concatenate([r["wemb"] for r in res.results]).reshape(BS, NT, D)
    # reference returns z + (emb[w] - z) (straight-through); replicate its
    # fp32 rounding exactly
    z = np.asarray(z_e, dtype=np.float32).reshape(BS, NT, D)
    w_emb = z + (w_emb.astype(np.float32) - z)
    return w.astype(np.int32), w_emb.astype(np.float32)


if __name__ == "__main__":
    d = np.load("/root/problem/inputs.npz")
    w, w_emb = kernel(d["z_e"], d["emb_table"])
    w_ref, wemb_ref = d["w"], d["w_emb"]
    nflip = int((w != w_ref).sum())
    rel = np.linalg.norm(w_emb - wemb_ref) / np.linalg.norm(wemb_ref)
    print(f"w mismatches: {nflip}/{w.size}")
    print(f"w_emb rel err: {rel:.3e}")


# revision 6
# speedup vs baseline: 1.1992x; 1.1992x over previous
"""VQ codebook nearest-neighbor kernel for Trainium2 (8 NeuronCores).

Problem: z_e (32,1024,64) f32, emb_table (8192,64) f32.
  dist2[t,v] = ||z_t||^2 - 2 z_t.e_v + ||e_v||^2
  w[t] = argmin_v dist2          (first index on ties)
  w_emb[t] = emb_table[w[t]]
Returns (w (32,1024) int32, w_emb (32,1024,64) f32).

Sharding: data-parallel over batch; core k handles batches [4k, 4k+4).
Per core: score[t,v] = 2 z.e_v - ||e_v||^2 (argmax == argmin dist) via
fp32 matmuls (K=65 incl. the -||e||^2 row) into PSUM, reduce-max on DVE,
ACT drains PSUM->SBUF, max_index recovers the argmax index exactly
(first-occurrence ties like jnp.argmin), then indirect DMA gathers
emb_table rows for w_emb.
"""
import sys

sys.path.insert(0, "/opt/trn_rl_repo")

import numpy as np

import concourse.bass as bass
import concourse.mybir as mybir
from concourse import bacc
from concourse.bass_utils import run_bass_kernel_spmd
from concourse.tile import TileContext

N_CORES = 8
BS, NT, D, V = 32, 1024, 64, 8192
TOK_PER_CORE = BS * NT // N_CORES          # 4096
P = 128                                    # tokens per tile (partitions)
N_TILES = TOK_PER_CORE // P                # 32
KAUG = D + 1                               # 65: z dims + ones row
QUARTER = 2048                             # psum quarter (4 banks)
N_Q = V // QUARTER                         # 4 quarters
CHUNK = 512                                # matmul free dim / psum bank
CPQ = QUARTER // CHUNK                     # chunks per quarter


def build_program(n_tiles=N_TILES, repeats=1, ablate=()):
    """ablate: subset of {'reduce','act','maxidx','gather'} to skip."""
    nc = bacc.Bacc("TRN2", target_bir_lowering=False)
    zt_d = nc.dram_tensor("zt", [KAUG, n_tiles * P], mybir.dt.float32,
                          kind="ExternalInput")
    b_d = nc.dram_tensor("baug", [KAUG, V], mybir.dt.float32,
                         kind="ExternalInput")
    emb_d = nc.dram_tensor("emb", [V, D], mybir.dt.float32,
                           kind="ExternalInput")
    w_d = nc.dram_tensor("w", [n_tiles * P, 1], mybir.dt.int32,
                         kind="ExternalOutput")
    wemb_d = nc.dram_tensor("wemb", [n_tiles * P, D], mybir.dt.float32,
                            kind="ExternalOutput")

    with TileContext(nc) as tc:
        with tc.tile_pool(name="bsb", bufs=1) as bsb, \
             tc.tile_pool(name="zsb", bufs=2) as zsb, \
             tc.tile_pool(name="ssb", bufs=2) as ssb, \
             tc.tile_pool(name="small", bufs=3) as small, \
             tc.tile_pool(name="wsb", bufs=3) as wsb, \
             tc.tile_pool(name="ps", bufs=2, space="PSUM") as ps:
            baug = bsb.tile([KAUG, V], mybir.dt.float32)
            nc.sync.dma_start(out=baug[:], in_=b_d[:])

            for t in [t for _ in range(repeats) for t in range(n_tiles)]:
                zt = zsb.tile([KAUG, P], mybir.dt.float32)
                nc.sync.dma_start(out=zt[:], in_=zt_d[:, t * P:(t + 1) * P])

                scores = ssb.tile([P, V], mybir.dt.float32)
                qmaxs = small.tile([P, N_Q], mybir.dt.float32)
                for q in range(N_Q):
                    pq = ps.tile([P, QUARTER], mybir.dt.float32)
                    for cc in range(CPQ):
                        c = q * CPQ + cc
                        nc.tensor.matmul(
                            out=pq[:, cc * CHUNK:(cc + 1) * CHUNK],
                            lhsT=zt[:],
                            rhs=baug[:, c * CHUNK:(c + 1) * CHUNK],
                            start=True, stop=True)
                    # chunk-max on DVE straight from PSUM
                    if 'reduce' not in ablate:
                        nc.vector.tensor_reduce(
                            out=qmaxs[:, q:q + 1], in_=pq[:],
                            axis=mybir.AxisListType.X, op=mybir.AluOpType.max)
                    # drain PSUM quarter to SBUF on ACT
                    if 'act' not in ablate:
                        nc.scalar.copy(out=scores[:, q * QUARTER:(q + 1) * QUARTER],
                                       in_=pq[:])
                    else:
                        nc.vector.tensor_copy(out=scores[:, q * QUARTER:q * QUARTER + 8],
                                              in_=pq[:, 0:8])

                gm8 = small.tile([P, 8], mybir.dt.float32)
                if 'reduce' not in ablate:
                    nc.vector.tensor_reduce(
                        out=gm8[:, 0:1], in_=qmaxs[:],
                        axis=mybir.AxisListType.X, op=mybir.AluOpType.max)
                else:
                    nc.vector.memset(gm8[:, 0:1], 0.0)
                nc.vector.tensor_copy(
                    out=gm8[:, 1:8],
                    in_=gm8[:, 0:1].to_broadcast([P, 7]))
                i8 = small.tile([P, 8], mybir.dt.uint32)
                if 'maxidx' not in ablate:
                    nc.vector.max_index(out=i8[:], in_max=gm8[:], in_values=scores[:])
                else:
                    nc.vector.memset(i8[:], 0)

                w_i32 = wsb.tile([P, 1], mybir.dt.int32)
                nc.vector.tensor_copy(out=w_i32[:], in_=i8[:, 0:1])
                nc.sync.dma_start(out=w_d[t * P:(t + 1) * P, :], in_=w_i32[:])

                if 'gather' not in ablate:
                    wemb = wsb.tile([P, D], mybir.dt.float32)
                    nc.gpsimd.indirect_dma_start(
                        out=wemb[:], out_offset=None, in_=emb_d[:],
                        in_offset=bass.IndirectOffsetOnAxis(ap=w_i32[:, :1], axis=0))
                    nc.sync.dma_start(out=wemb_d[t * P:(t + 1) * P, :], in_=wemb[:])

    nc.compile()
    return nc


def _prep_inputs(z_e, emb_table):
    """Host-side packing: per-core zT_aug shards + shared Baug."""
    z = np.ascontiguousarray(z_e, dtype=np.float32)
    E = np.ascontiguousarray(emb_table, dtype=np.float32)
    e_sq = (E * E).sum(axis=1, dtype=np.float32)
    baug = np.empty((KAUG, V), dtype=np.float32)
    baug[:D] = (2.0 * E).T
    baug[D] = -e_sq
    in_maps = []
    for k in range(N_CORES):
        zk = z.reshape(N_CORES, TOK_PER_CORE, D)[k]
        zt = np.empty((KAUG, TOK_PER_CORE), dtype=np.float32)
        zt[:D] = zk.T
        zt[D] = 1.0
        in_maps.append({"zt": zt, "baug": baug, "emb": E})
    return in_maps


_NC_CACHE = {}


def kernel(z_e, emb_table):
    if "nc" not in _NC_CACHE:
        _NC_CACHE["nc"] = build_program()
    nc = _NC_CACHE["nc"]
    in_maps = _prep_inputs(z_e, emb_table)
    res = run_bass_kernel_spmd(nc, in_maps, list(range(N_CORES)))
    w = np.concatenate([r["w"][:, 0] for r in res.results]).reshape(BS, NT)
    w_emb = np.concatenate([r["wemb"] for r in res.results]).reshape(BS, NT, D)
    # reference returns z + (emb[w] - z) (straight-through); replicate its
    # fp32 rounding exactly
    z = np.asarray(z_e, dtype=np.float32).reshape(BS, NT, D)
    w_emb = z + (w_emb.astype(np.float32) - z)
    return w.astype(np.int32), w_emb.astype(np.float32)


if __name__ == "__main__":
    d = np.load("/root/problem/inputs.npz")
    w, w_emb = kernel(d["z_e"], d["emb_table"])
    w_ref, wemb_ref = d["w"], d["w_emb"]
    nflip = int((w != w_ref).sum())
    rel = np.linalg.norm(w_emb - wemb_ref) / np.linalg.norm(wemb_ref)
    print(f"w mismatches: {nflip}/{w.size}")
    print(f"w_emb rel err: {rel:.3e}")


# revision 9
# speedup vs baseline: 1.2190x; 1.0166x over previous
"""VQ codebook nearest-neighbor kernel for Trainium2 (8 NeuronCores).

Problem: z_e (32,1024,64) f32, emb_table (8192,64) f32.
  dist2[t,v] = ||z_t||^2 - 2 z_t.e_v + ||e_v||^2
  w[t] = argmin_v dist2          (first index on ties)
  w_emb[t] = emb_table[w[t]]
Returns (w (32,1024) int32, w_emb (32,1024,64) f32).

Sharding: data-parallel over batch; core k handles batches [4k, 4k+4).
Per core: score[t,v] = 2 z.e_v - ||e_v||^2 (argmax == argmin dist) via
fp32 matmuls (K=65 incl. the -||e||^2 row) into PSUM, reduce-max on DVE,
ACT drains PSUM->SBUF, max_index recovers the argmax index exactly
(first-occurrence ties like jnp.argmin), then indirect DMA gathers
emb_table rows for w_emb.
"""
import sys

sys.path.insert(0, "/opt/trn_rl_repo")

import numpy as np

import concourse.bass as bass
import concourse.mybir as mybir
from concourse import bacc
from concourse.bass_utils import run_bass_kernel_spmd
from concourse.tile import TileContext

N_CORES = 8
BS, NT, D, V = 32, 1024, 64, 8192
TOK_PER_CORE = BS * NT // N_CORES          # 4096
P = 128                                    # tokens per tile (partitions)
N_TILES = TOK_PER_CORE // P                # 32
KAUG = D + 1                               # 65: z dims + ones row
QUARTER = 2048                             # psum quarter (4 banks)
N_Q = V // QUARTER                         # 4 quarters
CHUNK = 512                                # matmul free dim / psum bank
CPQ = QUARTER // CHUNK                     # chunks per quarter


def build_program(n_tiles=N_TILES, repeats=1, ablate=()):
    """ablate: subset of {'reduce','act','maxidx','gather'} to skip."""
    nc = bacc.Bacc("TRN2", target_bir_lowering=False)
    zt_d = nc.dram_tensor("zt", [KAUG, n_tiles * P], mybir.dt.float32,
                          kind="ExternalInput")
    b_d = nc.dram_tensor("baug", [KAUG, V], mybir.dt.float32,
                         kind="ExternalInput")
    emb_d = nc.dram_tensor("emb", [V, D], mybir.dt.float32,
                           kind="ExternalInput")
    w_d = nc.dram_tensor("w", [n_tiles * P, 1], mybir.dt.int32,
                         kind="ExternalOutput")
    wemb_d = nc.dram_tensor("wemb", [n_tiles * P, D], mybir.dt.float32,
                            kind="ExternalOutput")

    with TileContext(nc) as tc:
        with tc.tile_pool(name="bsb", bufs=1) as bsb, \
             tc.tile_pool(name="zsb", bufs=2) as zsb, \
             tc.tile_pool(name="ssb", bufs=2) as ssb, \
             tc.tile_pool(name="small", bufs=3) as small, \
             tc.tile_pool(name="wsb", bufs=3) as wsb, \
             tc.tile_pool(name="ps", bufs=2, space="PSUM") as ps:
            baug = bsb.tile([KAUG, V], mybir.dt.float32)
            nc.sync.dma_start(out=baug[:], in_=b_d[:])

            for t in [t for _ in range(repeats) for t in range(n_tiles)]:
                zt = zsb.tile([KAUG, P], mybir.dt.float32)
                nc.sync.dma_start(out=zt[:], in_=zt_d[:, t * P:(t + 1) * P])

                scores = ssb.tile([P, V], mybir.dt.float32)
                qmaxs = small.tile([P, N_Q], mybir.dt.float32)
                for q in range(N_Q):
                    pq = ps.tile([P, QUARTER], mybir.dt.float32)
                    for cc in range(CPQ):
                        c = q * CPQ + cc
                        nc.tensor.matmul(
                            out=pq[:, cc * CHUNK:(cc + 1) * CHUNK],
                            lhsT=zt[:],
                            rhs=baug[:, c * CHUNK:(c + 1) * CHUNK],
                            start=True, stop=True)
                    # chunk-max on DVE straight from PSUM
                    if 'reduce' not in ablate:
                        nc.vector.tensor_reduce(
                            out=qmaxs[:, q:q + 1], in_=pq[:],
                            axis=mybir.AxisListType.X, op=mybir.AluOpType.max)
                    # drain PSUM quarter to SBUF on ACT
                    if 'act' not in ablate:
                        nc.scalar.copy(out=scores[:, q * QUARTER:(q + 1) * QUARTER],
                                       in_=pq[:])
                    else:
                        nc.vector.tensor_copy(out=scores[:, q * QUARTER:q * QUARTER + 8],
                                              in_=pq[:, 0:8])

                gm8 = small.tile([P, 8], mybir.dt.float32)
                if 'reduce' not in ablate:
                    nc.vector.tensor_reduce(
                        out=gm8[:, 0:1], in_=qmaxs[:],
                        axis=mybir.AxisListType.X, op=mybir.AluOpType.max)
                else:
                    nc.vector.memset(gm8[:, 0:1], 0.0)
                nc.vector.tensor_copy(
                    out=gm8[:, 1:8],
                    in_=gm8[:, 0:1].to_broadcast([P, 7]))
                i8 = small.tile([P, 8], mybir.dt.uint32)
                if 'maxidx' not in ablate:
                    nc.vector.max_index(out=i8[:], in_max=gm8[:], in_values=scores[:])
                else:
                    nc.vector.memset(i8[:], 0)

                w_i32 = wsb.tile([P, 1], mybir.dt.int32)
                nc.vector.tensor_copy(out=w_i32[:], in_=i8[:, 0:1])
                nc.sync.dma_start(out=w_d[t * P:(t + 1) * P, :], in_=w_i32[:])

                if 'gather' not in ablate:
                    wemb = wsb.tile([P, D], mybir.dt.float32)
                    nc.gpsimd.indirect_dma_start(
                        out=wemb[:], out_offset=None, in_=emb_d[:],
                        in_offset=bass.IndirectOffsetOnAxis(ap=w_i32[:, :1], axis=0))
                    nc.sync.dma_start(out=wemb_d[t * P:(t + 1) * P, :], in_=wemb[:])

    nc.compile()
    return nc


SUB = 8                                    # sub-chunk size for the pyramid
N_SUB = V // SUB                           # 1024 sub-chunks
ETAB_W = 128                               # padded rescore-table row width


def build_program_v2(n_tiles=N_TILES, repeats=1, debug=False):
    """fp16 limb-pair matmuls + sub-chunk max pyramid + 8-candidate exact
    rescore via indirect DMA gather. No ACT drain, no full-row max_index."""
    nc = bacc.Bacc("TRN2", target_bir_lowering=False)
    f16, f32 = mybir.dt.float16, mybir.dt.float32
    ntok = n_tiles * P
    zl_d = nc.dram_tensor("zl", [2 * D, ntok], f16, kind="ExternalInput")
    bh_d = nc.dram_tensor("bh", [2 * D, V], f16, kind="ExternalInput")   # [B0;B1]
    bx_d = nc.dram_tensor("bx", [2 * D, V], f16, kind="ExternalInput")   # [B1;B0]
    esq_d = nc.dram_tensor("esql", [4, V], f16, kind="ExternalInput")    # 3 limbs+0
    zr_d = nc.dram_tensor("zrow", [ntok, ETAB_W], f32, kind="ExternalInput")
    et_d = nc.dram_tensor("etab", [V, ETAB_W], f32, kind="ExternalInput")
    emb_d = nc.dram_tensor("emb", [V, D], f32, kind="ExternalInput")
    w_d = nc.dram_tensor("w", [ntok, 1], mybir.dt.int32, kind="ExternalOutput")
    wemb_d = nc.dram_tensor("wemb", [ntok, D], f32, kind="ExternalOutput")
    if debug:
        cm8_d = nc.dram_tensor("cm8_dbg", [ntok, N_SUB], f32, kind="ExternalOutput")
        g8_d = nc.dram_tensor("g8_dbg", [ntok, 8], mybir.dt.uint32, kind="ExternalOutput")
        offs_d = nc.dram_tensor("offs_dbg", [ntok, SUB], mybir.dt.int32, kind="ExternalOutput")
        s8_d = nc.dram_tensor("s8_dbg", [ntok, SUB], f32, kind="ExternalOutput")
        cand_d = nc.dram_tensor("cand_dbg", [ntok, SUB * ETAB_W], f32, kind="ExternalOutput")

    with TileContext(nc) as tc:
        with tc.tile_pool(name="cbsb", bufs=1) as cbsb, \
             tc.tile_pool(name="zsb", bufs=3) as zsb, \
             tc.tile_pool(name="small", bufs=3) as small, \
             tc.tile_pool(name="csb", bufs=2) as csb, \
             tc.tile_pool(name="wsb", bufs=3) as wsb, \
             tc.tile_pool(name="ps", bufs=2, space="PSUM") as ps:
            bh = cbsb.tile([2 * D, V], f16)
            nc.sync.dma_start(out=bh[:], in_=bh_d[:])
            bx = cbsb.tile([2 * D, V], f16)
            nc.sync.dma_start(out=bx[:], in_=bx_d[:])
            esql = cbsb.tile([4, V], f16)
            nc.sync.dma_start(out=esql[:], in_=esq_d[:])
            ones4 = cbsb.tile([4, P], f16)
            nc.vector.memset(ones4[:], 1.0)
            jiota = cbsb.tile([P, SUB], f32)      # j
            # jneg = j - 16 (small bias keeps fp32 exact; mask*jneg stays <0)
            jneg = cbsb.tile([P, SUB], f32)
            for j in range(SUB):
                nc.vector.memset(jiota[:, j:j + 1], float(j))
                nc.vector.memset(jneg[:, j:j + 1], float(j) - 16.0)

            for t in [t for _ in range(repeats) for t in range(n_tiles)]:
                zl = zsb.tile([2 * D, P], f16)
                nc.sync.dma_start(out=zl[:], in_=zl_d[:, t * P:(t + 1) * P])
                zrow = zsb.tile([P, ETAB_W], f32)
                nc.sync.dma_start(out=zrow[:], in_=zr_d[t * P:(t + 1) * P, :])

                cm8 = csb.tile([P, N_SUB], f32)
                for q in range(N_Q):
                    pq = ps.tile([P, QUARTER], f32)
                    for cc in range(CPQ):
                        c = q * CPQ + cc
                        sl = slice(c * CHUNK, (c + 1) * CHUNK)
                        po = pq[:, cc * CHUNK:(cc + 1) * CHUNK]
                        nc.tensor.matmul(out=po, lhsT=ones4[:], rhs=esql[:, sl],
                                         start=True, stop=False)
                    for cc in range(CPQ):
                        c = q * CPQ + cc
                        sl = slice(c * CHUNK, (c + 1) * CHUNK)
                        po = pq[:, cc * CHUNK:(cc + 1) * CHUNK]
                        nc.tensor.matmul(out=po, lhsT=zl[:], rhs=bh[:, sl],
                                         start=False, stop=False)
                        nc.tensor.matmul(out=po, lhsT=zl[:], rhs=bx[:, sl],
                                         start=False, stop=True)
                    nc.vector.tensor_reduce(
                        out=cm8[:, q * (QUARTER // SUB):(q + 1) * (QUARTER // SUB)],
                        in_=pq[:].rearrange("p (c j) -> p c j", j=SUB),
                        axis=mybir.AxisListType.X, op=mybir.AluOpType.max)

                m8 = small.tile([P, 8], f32)
                g8 = small.tile([P, 8], mybir.dt.uint32)
                nc.vector.max(out=m8[:], in_=cm8[:])
                nc.vector.max_index(out=g8[:], in_max=m8[:], in_values=cm8[:])

                gf = small.tile([P, 1], f32)
                nc.vector.tensor_copy(out=gf[:], in_=g8[:, 0:1])
                base = small.tile([P, 1], f32)       # g* * 8
                nc.vector.tensor_scalar(out=base[:], in0=gf[:], scalar1=float(SUB),
                                        scalar2=None, op0=mybir.AluOpType.mult)
                offs_f = small.tile([P, SUB], f32)
                nc.vector.tensor_scalar(out=offs_f[:], in0=jiota[:],
                                        scalar1=base[:, 0:1], scalar2=None,
                                        op0=mybir.AluOpType.add)
                offs = small.tile([P, SUB], mybir.dt.int32)
                nc.vector.tensor_copy(out=offs[:], in_=offs_f[:])

                cand = csb.tile([P, SUB, ETAB_W], f32)
                nc.gpsimd.indirect_dma_start(
                    out=cand[:], out_offset=None, in_=et_d[:],
                    in_offset=bass.IndirectOffsetOnAxis(ap=offs[:, :], axis=0))

                prod = csb.tile([P, SUB * ETAB_W], f32)
                nc.vector.tensor_tensor(
                    out=prod[:], in0=cand[:],
                    in1=zrow[:, None, :].to_broadcast([P, SUB, ETAB_W]),
                    op=mybir.AluOpType.mult)
                s8 = small.tile([P, SUB], f32)
                nc.vector.tensor_reduce(
                    out=s8[:], in_=prod[:].rearrange("p (c j) -> p c j", j=ETAB_W),
                    axis=mybir.AxisListType.X, op=mybir.AluOpType.add)

                sm = small.tile([P, 1], f32)
                nc.vector.tensor_reduce(out=sm[:], in_=s8[:],
                                        axis=mybir.AxisListType.X,
                                        op=mybir.AluOpType.max)
                mask = small.tile([P, SUB], f32)
                nc.vector.tensor_tensor(out=mask[:], in0=s8[:],
                                        in1=sm[:, 0:1].to_broadcast([P, SUB]),
                                        op=mybir.AluOpType.is_ge)
                msel = small.tile([P, SUB], f32)
                nc.vector.tensor_tensor(out=msel[:], in0=mask[:], in1=jneg[:],
                                        op=mybir.AluOpType.mult)
                jm = small.tile([P, 1], f32)
                nc.vector.tensor_reduce(out=jm[:], in_=msel[:],
                                        axis=mybir.AxisListType.X,
                                        op=mybir.AluOpType.min)
                w_f = small.tile([P, 1], f32)
                nc.vector.tensor_scalar(out=w_f[:], in0=jm[:], scalar1=16.0,
                                        scalar2=base[:, 0:1],
                                        op0=mybir.AluOpType.add,
                                        op1=mybir.AluOpType.add)
                w_i32 = wsb.tile([P, 1], mybir.dt.int32)
                nc.vector.tensor_copy(out=w_i32[:], in_=w_f[:])
                nc.sync.dma_start(out=w_d[t * P:(t + 1) * P, :], in_=w_i32[:])
                if debug:
                    nc.sync.dma_start(out=cm8_d[t * P:(t + 1) * P, :], in_=cm8[:])
                    nc.sync.dma_start(out=g8_d[t * P:(t + 1) * P, :], in_=g8[:])
                    nc.sync.dma_start(out=offs_d[t * P:(t + 1) * P, :], in_=offs[:])
                    nc.sync.dma_start(out=s8_d[t * P:(t + 1) * P, :], in_=s8[:])
                    nc.sync.dma_start(out=cand_d[t * P:(t + 1) * P, :], in_=prod[:])

                wemb = wsb.tile([P, D], f32)
                nc.gpsimd.indirect_dma_start(
                    out=wemb[:], out_offset=None, in_=emb_d[:],
                    in_offset=bass.IndirectOffsetOnAxis(ap=w_i32[:, :1], axis=0))
                nc.sync.dma_start(out=wemb_d[t * P:(t + 1) * P, :], in_=wemb[:])

    nc.compile()
    return nc


def _prep_inputs_v2(z_e, emb_table):
    z = np.ascontiguousarray(z_e, dtype=np.float32)
    E = np.ascontiguousarray(emb_table, dtype=np.float32)
    e_sq = (E * E).sum(axis=1, dtype=np.float32)
    B = (2.0 * E).T                                    # (64, V) f32
    B0 = B.astype(np.float16)
    B1 = (B - B0.astype(np.float32)).astype(np.float16)
    bh = np.concatenate([B0, B1], axis=0)              # (128, V)
    bx = np.concatenate([B1, B0], axis=0)
    nesq = -e_sq
    e0 = nesq.astype(np.float16)
    e1 = (nesq - e0.astype(np.float32)).astype(np.float16)
    e2 = (nesq - e0.astype(np.float32) - e1.astype(np.float32)).astype(np.float16)
    esql = np.stack([e0, e1, e2, np.zeros_like(e0)], axis=0)  # (4, V)
    etab = np.zeros((V, ETAB_W), dtype=np.float32)
    etab[:, :D] = 2.0 * E
    etab[:, D] = -e_sq
    in_maps = []
    for k in range(N_CORES):
        zk = z.reshape(N_CORES, TOK_PER_CORE, D)[k]    # (ntok, 64)
        zt = zk.T                                      # (64, ntok) f32
        Z0 = zt.astype(np.float16)
        Z1 = (zt - Z0.astype(np.float32)).astype(np.float16)
        zl = np.concatenate([Z0, Z1], axis=0)          # (128, ntok)
        zrow = np.zeros((TOK_PER_CORE, ETAB_W), dtype=np.float32)
        zrow[:, :D] = zk
        zrow[:, D] = 1.0
        in_maps.append({"zl": zl, "bh": bh, "bx": bx, "esql": esql,
                        "zrow": zrow, "etab": etab, "emb": E})
    return in_maps


def _prep_inputs(z_e, emb_table):
    """Host-side packing: per-core zT_aug shards + shared Baug."""
    z = np.ascontiguousarray(z_e, dtype=np.float32)
    E = np.ascontiguousarray(emb_table, dtype=np.float32)
    e_sq = (E * E).sum(axis=1, dtype=np.float32)
    baug = np.empty((KAUG, V), dtype=np.float32)
    baug[:D] = (2.0 * E).T
    baug[D] = -e_sq
    in_maps = []
    for k in range(N_CORES):
        zk = z.reshape(N_CORES, TOK_PER_CORE, D)[k]
        zt = np.empty((KAUG, TOK_PER_CORE), dtype=np.float32)
        zt[:D] = zk.T
        zt[D] = 1.0
        in_maps.append({"zt": zt, "baug": baug, "emb": E})
    return in_maps


_NC_CACHE = {}


def kernel(z_e, emb_table):
    if "nc" not in _NC_CACHE:
        _NC_CACHE["nc"] = build_program()
    nc = _NC_CACHE["nc"]
    in_maps = _prep_inputs(z_e, emb_table)
    res = run_bass_kernel_spmd(nc, in_maps, list(range(N_CORES)))
    w = np.concatenate([r["w"][:, 0] for r in res.results]).reshape(BS, NT)
    w_emb = np.concatenate([r["wemb"] for r in res.results]).reshape(BS, NT, D)
    # reference returns z + (emb[w] - z) (straight-through); replicate its
    # fp32 rounding exactly
    z = np.asarray(z_e, dtype=np.float32).reshape(BS, NT, D)
    w_emb = z + (w_emb.astype(np.float32) - z)
    return w.astype(np.int32), w_emb.astype(np.float32)


if __name__ == "__main__":
    d = np.load("/root/problem/inputs.npz")
    w, w_emb = kernel(d["z_e"], d["emb_table"])
    w_ref, wemb_ref = d["w"], d["w_emb"]
    nflip = int((w != w_ref).sum())
    rel = np.linalg.norm(w_emb - wemb_ref) / np.linalg.norm(wemb_ref)
    print(f"w mismatches: {nflip}/{w.size}")
    print(f"w_emb rel err: {rel:.3e}")


# revision 11
# speedup vs baseline: 2.5449x; 2.0876x over previous
"""VQ codebook nearest-neighbor kernel for Trainium2 (8 NeuronCores).

Problem: z_e (32,1024,64) f32, emb_table (8192,64) f32.
  dist2[t,v] = ||z_t||^2 - 2 z_t.e_v + ||e_v||^2
  w[t] = argmin_v dist2          (first index on ties)
  w_emb[t] = emb_table[w[t]]
Returns (w (32,1024) int32, w_emb (32,1024,64) f32).

Sharding: data-parallel over batch; core k handles batches [4k, 4k+4).
Per core: score[t,v] = 2 z.e_v - ||e_v||^2 (argmax == argmin dist) via
fp32 matmuls (K=65 incl. the -||e||^2 row) into PSUM, reduce-max on DVE,
ACT drains PSUM->SBUF, max_index recovers the argmax index exactly
(first-occurrence ties like jnp.argmin), then indirect DMA gathers
emb_table rows for w_emb.
"""
import sys

sys.path.insert(0, "/opt/trn_rl_repo")

import numpy as np

import concourse.bass as bass
import concourse.mybir as mybir
from concourse import bacc
from concourse.bass_utils import run_bass_kernel_spmd
from concourse.tile import TileContext

N_CORES = 8
BS, NT, D, V = 32, 1024, 64, 8192
TOK_PER_CORE = BS * NT // N_CORES          # 4096
P = 128                                    # tokens per tile (partitions)
N_TILES = TOK_PER_CORE // P                # 32
KAUG = D + 1                               # 65: z dims + ones row
QUARTER = 2048                             # psum quarter (4 banks)
N_Q = V // QUARTER                         # 4 quarters
CHUNK = 512                                # matmul free dim / psum bank
CPQ = QUARTER // CHUNK                     # chunks per quarter


def build_program(n_tiles=N_TILES, repeats=1, ablate=()):
    """ablate: subset of {'reduce','act','maxidx','gather'} to skip."""
    nc = bacc.Bacc("TRN2", target_bir_lowering=False)
    zt_d = nc.dram_tensor("zt", [KAUG, n_tiles * P], mybir.dt.float32,
                          kind="ExternalInput")
    b_d = nc.dram_tensor("baug", [KAUG, V], mybir.dt.float32,
                         kind="ExternalInput")
    emb_d = nc.dram_tensor("emb", [V, D], mybir.dt.float32,
                           kind="ExternalInput")
    w_d = nc.dram_tensor("w", [n_tiles * P, 1], mybir.dt.int32,
                         kind="ExternalOutput")
    wemb_d = nc.dram_tensor("wemb", [n_tiles * P, D], mybir.dt.float32,
                            kind="ExternalOutput")

    with TileContext(nc) as tc:
        with tc.tile_pool(name="bsb", bufs=1) as bsb, \
             tc.tile_pool(name="zsb", bufs=2) as zsb, \
             tc.tile_pool(name="ssb", bufs=2) as ssb, \
             tc.tile_pool(name="small", bufs=3) as small, \
             tc.tile_pool(name="wsb", bufs=3) as wsb, \
             tc.tile_pool(name="ps", bufs=2, space="PSUM") as ps:
            baug = bsb.tile([KAUG, V], mybir.dt.float32)
            nc.sync.dma_start(out=baug[:], in_=b_d[:])

            for t in [t for _ in range(repeats) for t in range(n_tiles)]:
                zt = zsb.tile([KAUG, P], mybir.dt.float32)
                nc.sync.dma_start(out=zt[:], in_=zt_d[:, t * P:(t + 1) * P])

                scores = ssb.tile([P, V], mybir.dt.float32)
                qmaxs = small.tile([P, N_Q], mybir.dt.float32)
                for q in range(N_Q):
                    pq = ps.tile([P, QUARTER], mybir.dt.float32)
                    for cc in range(CPQ):
                        c = q * CPQ + cc
                        nc.tensor.matmul(
                            out=pq[:, cc * CHUNK:(cc + 1) * CHUNK],
                            lhsT=zt[:],
                            rhs=baug[:, c * CHUNK:(c + 1) * CHUNK],
                            start=True, stop=True)
                    # chunk-max on DVE straight from PSUM
                    if 'reduce' not in ablate:
                        nc.vector.tensor_reduce(
                            out=qmaxs[:, q:q + 1], in_=pq[:],
                            axis=mybir.AxisListType.X, op=mybir.AluOpType.max)
                    # drain PSUM quarter to SBUF on ACT
                    if 'act' not in ablate:
                        nc.scalar.copy(out=scores[:, q * QUARTER:(q + 1) * QUARTER],
                                       in_=pq[:])
                    else:
                        nc.vector.tensor_copy(out=scores[:, q * QUARTER:q * QUARTER + 8],
                                              in_=pq[:, 0:8])

                gm8 = small.tile([P, 8], mybir.dt.float32)
                if 'reduce' not in ablate:
                    nc.vector.tensor_reduce(
                        out=gm8[:, 0:1], in_=qmaxs[:],
                        axis=mybir.AxisListType.X, op=mybir.AluOpType.max)
                else:
                    nc.vector.memset(gm8[:, 0:1], 0.0)
                nc.vector.tensor_copy(
                    out=gm8[:, 1:8],
                    in_=gm8[:, 0:1].to_broadcast([P, 7]))
                i8 = small.tile([P, 8], mybir.dt.uint32)
                if 'maxidx' not in ablate:
                    nc.vector.max_index(out=i8[:], in_max=gm8[:], in_values=scores[:])
                else:
                    nc.vector.memset(i8[:], 0)

                w_i32 = wsb.tile([P, 1], mybir.dt.int32)
                nc.vector.tensor_copy(out=w_i32[:], in_=i8[:, 0:1])
                nc.sync.dma_start(out=w_d[t * P:(t + 1) * P, :], in_=w_i32[:])

                if 'gather' not in ablate:
                    wemb = wsb.tile([P, D], mybir.dt.float32)
                    nc.gpsimd.indirect_dma_start(
                        out=wemb[:], out_offset=None, in_=emb_d[:],
                        in_offset=bass.IndirectOffsetOnAxis(ap=w_i32[:, :1], axis=0))
                    nc.sync.dma_start(out=wemb_d[t * P:(t + 1) * P, :], in_=wemb[:])

    nc.compile()
    return nc


SUB = 8                                    # sub-chunk size for the pyramid
N_SUB = V // SUB                           # 1024 sub-chunks
ETAB_W = 128                               # padded rescore-table row width


def build_program_v2(n_tiles=N_TILES, repeats=1, debug=False):
    """fp16 limb-pair matmuls + sub-chunk max pyramid + 8-candidate exact
    rescore via indirect DMA gather. No ACT drain, no full-row max_index."""
    nc = bacc.Bacc("TRN2", target_bir_lowering=False)
    f16, f32 = mybir.dt.float16, mybir.dt.float32
    ntok = n_tiles * P
    zl_d = nc.dram_tensor("zl", [2 * D, ntok], f16, kind="ExternalInput")
    bh_d = nc.dram_tensor("bh", [2 * D, V], f16, kind="ExternalInput")   # [B0;B1]
    bx_d = nc.dram_tensor("bx", [2 * D, V], f16, kind="ExternalInput")   # [B1;B0]
    esq_d = nc.dram_tensor("esql", [4, V], f16, kind="ExternalInput")    # 3 limbs+0
    zr_d = nc.dram_tensor("zrow", [ntok, ETAB_W], f32, kind="ExternalInput")
    et_d = nc.dram_tensor("etab", [V, ETAB_W], f32, kind="ExternalInput")
    emb_d = nc.dram_tensor("emb", [V, D], f32, kind="ExternalInput")
    w_d = nc.dram_tensor("w", [ntok, 1], mybir.dt.int32, kind="ExternalOutput")
    wemb_d = nc.dram_tensor("wemb", [ntok, D], f32, kind="ExternalOutput")
    if debug:
        cm8_d = nc.dram_tensor("cm8_dbg", [ntok, N_SUB], f32, kind="ExternalOutput")
        g8_d = nc.dram_tensor("g8_dbg", [ntok, 8], mybir.dt.uint32, kind="ExternalOutput")
        offs_d = nc.dram_tensor("offs_dbg", [ntok, SUB], mybir.dt.int32, kind="ExternalOutput")
        s8_d = nc.dram_tensor("s8_dbg", [ntok, SUB], f32, kind="ExternalOutput")
        cand_d = nc.dram_tensor("cand_dbg", [ntok, SUB * ETAB_W], f32, kind="ExternalOutput")

    with TileContext(nc) as tc:
        with tc.tile_pool(name="cbsb", bufs=1) as cbsb, \
             tc.tile_pool(name="zsb", bufs=3) as zsb, \
             tc.tile_pool(name="small", bufs=3) as small, \
             tc.tile_pool(name="csb", bufs=2) as csb, \
             tc.tile_pool(name="wsb", bufs=3) as wsb, \
             tc.tile_pool(name="ps", bufs=2, space="PSUM") as ps:
            bh = cbsb.tile([2 * D, V], f16)
            nc.sync.dma_start(out=bh[:], in_=bh_d[:])
            bx = cbsb.tile([2 * D, V], f16)
            nc.sync.dma_start(out=bx[:], in_=bx_d[:])
            esql = cbsb.tile([4, V], f16)
            nc.sync.dma_start(out=esql[:], in_=esq_d[:])
            ones4 = cbsb.tile([4, P], f16)
            nc.vector.memset(ones4[:], 1.0)
            jiota = cbsb.tile([P, SUB], f32)      # j
            # jneg = j - 16 (small bias keeps fp32 exact; mask*jneg stays <0)
            jneg = cbsb.tile([P, SUB], f32)
            for j in range(SUB):
                nc.vector.memset(jiota[:, j:j + 1], float(j))
                nc.vector.memset(jneg[:, j:j + 1], float(j) - 16.0)

            for t in [t for _ in range(repeats) for t in range(n_tiles)]:
                zl = zsb.tile([2 * D, P], f16)
                nc.sync.dma_start(out=zl[:], in_=zl_d[:, t * P:(t + 1) * P])
                zrow = zsb.tile([P, ETAB_W], f32)
                nc.sync.dma_start(out=zrow[:], in_=zr_d[t * P:(t + 1) * P, :])

                cm8 = csb.tile([P, N_SUB], f32)
                for q in range(N_Q):
                    pq = ps.tile([P, QUARTER], f32)
                    for cc in range(CPQ):
                        c = q * CPQ + cc
                        sl = slice(c * CHUNK, (c + 1) * CHUNK)
                        po = pq[:, cc * CHUNK:(cc + 1) * CHUNK]
                        nc.tensor.matmul(out=po, lhsT=ones4[:], rhs=esql[:, sl],
                                         start=True, stop=False)
                    for cc in range(CPQ):
                        c = q * CPQ + cc
                        sl = slice(c * CHUNK, (c + 1) * CHUNK)
                        po = pq[:, cc * CHUNK:(cc + 1) * CHUNK]
                        nc.tensor.matmul(out=po, lhsT=zl[:], rhs=bh[:, sl],
                                         start=False, stop=False)
                        nc.tensor.matmul(out=po, lhsT=zl[:], rhs=bx[:, sl],
                                         start=False, stop=True)
                    nc.vector.tensor_reduce(
                        out=cm8[:, q * (QUARTER // SUB):(q + 1) * (QUARTER // SUB)],
                        in_=pq[:].rearrange("p (c j) -> p c j", j=SUB),
                        axis=mybir.AxisListType.X, op=mybir.AluOpType.max)

                m8 = small.tile([P, 8], f32)
                g8 = small.tile([P, 8], mybir.dt.uint32)
                nc.vector.max(out=m8[:], in_=cm8[:])
                nc.vector.max_index(out=g8[:], in_max=m8[:], in_values=cm8[:])

                gf = small.tile([P, 1], f32)
                nc.vector.tensor_copy(out=gf[:], in_=g8[:, 0:1])
                base = small.tile([P, 1], f32)       # g* * 8
                nc.vector.tensor_scalar(out=base[:], in0=gf[:], scalar1=float(SUB),
                                        scalar2=None, op0=mybir.AluOpType.mult)
                offs_f = small.tile([P, SUB], f32)
                nc.vector.tensor_scalar(out=offs_f[:], in0=jiota[:],
                                        scalar1=base[:, 0:1], scalar2=None,
                                        op0=mybir.AluOpType.add)
                offs = small.tile([P, SUB], mybir.dt.int32)
                nc.vector.tensor_copy(out=offs[:], in_=offs_f[:])

                cand = csb.tile([P, SUB, ETAB_W], f32)
                # HW consumes multi-offset APs in a different order than the
                # sim — issue one single-offset gather per candidate row
                for j in range(SUB):
                    nc.gpsimd.indirect_dma_start(
                        out=cand[:, j, :], out_offset=None, in_=et_d[:],
                        in_offset=bass.IndirectOffsetOnAxis(ap=offs[:, j:j + 1], axis=0))

                prod = csb.tile([P, SUB * ETAB_W], f32)
                nc.vector.tensor_tensor(
                    out=prod[:], in0=cand[:],
                    in1=zrow[:, None, :].to_broadcast([P, SUB, ETAB_W]),
                    op=mybir.AluOpType.mult)
                s8 = small.tile([P, SUB], f32)
                nc.vector.tensor_reduce(
                    out=s8[:], in_=prod[:].rearrange("p (c j) -> p c j", j=ETAB_W),
                    axis=mybir.AxisListType.X, op=mybir.AluOpType.add)

                sm = small.tile([P, 1], f32)
                nc.vector.tensor_reduce(out=sm[:], in_=s8[:],
                                        axis=mybir.AxisListType.X,
                                        op=mybir.AluOpType.max)
                mask = small.tile([P, SUB], f32)
                nc.vector.tensor_tensor(out=mask[:], in0=s8[:],
                                        in1=sm[:, 0:1].to_broadcast([P, SUB]),
                                        op=mybir.AluOpType.is_ge)
                msel = small.tile([P, SUB], f32)
                nc.vector.tensor_tensor(out=msel[:], in0=mask[:], in1=jneg[:],
                                        op=mybir.AluOpType.mult)
                jm = small.tile([P, 1], f32)
                nc.vector.tensor_reduce(out=jm[:], in_=msel[:],
                                        axis=mybir.AxisListType.X,
                                        op=mybir.AluOpType.min)
                w_f = small.tile([P, 1], f32)
                nc.vector.tensor_scalar(out=w_f[:], in0=jm[:], scalar1=16.0,
                                        scalar2=base[:, 0:1],
                                        op0=mybir.AluOpType.add,
                                        op1=mybir.AluOpType.add)
                w_i32 = wsb.tile([P, 1], mybir.dt.int32)
                nc.vector.tensor_copy(out=w_i32[:], in_=w_f[:])
                nc.sync.dma_start(out=w_d[t * P:(t + 1) * P, :], in_=w_i32[:])
                if debug:
                    nc.sync.dma_start(out=cm8_d[t * P:(t + 1) * P, :], in_=cm8[:])
                    nc.sync.dma_start(out=g8_d[t * P:(t + 1) * P, :], in_=g8[:])
                    nc.sync.dma_start(out=offs_d[t * P:(t + 1) * P, :], in_=offs[:])
                    nc.sync.dma_start(out=s8_d[t * P:(t + 1) * P, :], in_=s8[:])
                    nc.sync.dma_start(out=cand_d[t * P:(t + 1) * P, :], in_=cand[:])

                wemb = wsb.tile([P, D], f32)
                nc.gpsimd.indirect_dma_start(
                    out=wemb[:], out_offset=None, in_=emb_d[:],
                    in_offset=bass.IndirectOffsetOnAxis(ap=w_i32[:, :1], axis=0))
                nc.sync.dma_start(out=wemb_d[t * P:(t + 1) * P, :], in_=wemb[:])

    nc.compile()
    return nc


def _prep_inputs_v2(z_e, emb_table):
    z = np.ascontiguousarray(z_e, dtype=np.float32)
    E = np.ascontiguousarray(emb_table, dtype=np.float32)
    e_sq = (E * E).sum(axis=1, dtype=np.float32)
    B = (2.0 * E).T                                    # (64, V) f32
    B0 = B.astype(np.float16)
    B1 = (B - B0.astype(np.float32)).astype(np.float16)
    bh = np.concatenate([B0, B1], axis=0)              # (128, V)
    bx = np.concatenate([B1, B0], axis=0)
    nesq = -e_sq
    e0 = nesq.astype(np.float16)
    e1 = (nesq - e0.astype(np.float32)).astype(np.float16)
    e2 = (nesq - e0.astype(np.float32) - e1.astype(np.float32)).astype(np.float16)
    esql = np.stack([e0, e1, e2, np.zeros_like(e0)], axis=0)  # (4, V)
    etab = np.zeros((V, ETAB_W), dtype=np.float32)
    etab[:, :D] = 2.0 * E
    etab[:, D] = -e_sq
    in_maps = []
    for k in range(N_CORES):
        zk = z.reshape(N_CORES, TOK_PER_CORE, D)[k]    # (ntok, 64)
        zt = zk.T                                      # (64, ntok) f32
        Z0 = zt.astype(np.float16)
        Z1 = (zt - Z0.astype(np.float32)).astype(np.float16)
        zl = np.concatenate([Z0, Z1], axis=0)          # (128, ntok)
        zrow = np.zeros((TOK_PER_CORE, ETAB_W), dtype=np.float32)
        zrow[:, :D] = zk
        zrow[:, D] = 1.0
        in_maps.append({"zl": zl, "bh": bh, "bx": bx, "esql": esql,
                        "zrow": zrow, "etab": etab, "emb": E})
    return in_maps


def _prep_inputs(z_e, emb_table):
    """Host-side packing: per-core zT_aug shards + shared Baug."""
    z = np.ascontiguousarray(z_e, dtype=np.float32)
    E = np.ascontiguousarray(emb_table, dtype=np.float32)
    e_sq = (E * E).sum(axis=1, dtype=np.float32)
    baug = np.empty((KAUG, V), dtype=np.float32)
    baug[:D] = (2.0 * E).T
    baug[D] = -e_sq
    in_maps = []
    for k in range(N_CORES):
        zk = z.reshape(N_CORES, TOK_PER_CORE, D)[k]
        zt = np.empty((KAUG, TOK_PER_CORE), dtype=np.float32)
        zt[:D] = zk.T
        zt[D] = 1.0
        in_maps.append({"zt": zt, "baug": baug, "emb": E})
    return in_maps


_NC_CACHE = {}


def kernel(z_e, emb_table):
    if "nc" not in _NC_CACHE:
        _NC_CACHE["nc"] = build_program()
    nc = _NC_CACHE["nc"]
    in_maps = _prep_inputs(z_e, emb_table)
    res = run_bass_kernel_spmd(nc, in_maps, list(range(N_CORES)))
    w = np.concatenate([r["w"][:, 0] for r in res.results]).reshape(BS, NT)
    w_emb = np.concatenate([r["wemb"] for r in res.results]).reshape(BS, NT, D)
    # reference returns z + (emb[w] - z) (straight-through); replicate its
    # fp32 rounding exactly
    z = np.asarray(z_e, dtype=np.float32).reshape(BS, NT, D)
    w_emb = z + (w_emb.astype(np.float32) - z)
    return w.astype(np.int32), w_emb.astype(np.float32)


if __name__ == "__main__":
    d = np.load("/root/problem/inputs.npz")
    w, w_emb = kernel(d["z_e"], d["emb_table"])
    w_ref, wemb_ref = d["w"], d["w_emb"]
    nflip = int((w != w_ref).sum())
    rel = np.linalg.norm(w_emb - wemb_ref) / np.linalg.norm(wemb_ref)
    print(f"w mismatches: {nflip}/{w.size}")
    print(f"w_emb rel err: {rel:.3e}")
